# revision 5
# baseline (speedup 1.0000x reference)
"""Trainium2 Bass kernel for nn_HeatmapEncoder.

Math: for each (b, s, c) and each of two coordinate sets (gaze, hand):
    g = exp(-((gx-cx)^2 + (gy-cy)^2) / (2 sigma^2))   on a 336x336 grid
    g = g / (sum(g) + eps)            (zeroed when cx+cy <= 0)
    unified = g_gaze + g_hand
    out = unified / (max(unified) + eps)

Key structure: the Gaussian is separable, so each unified map is rank-2:
    unified = fy_g (x) fx_g * a_g  +  fy_h (x) fx_h * a_h
i.e. one K=2 matmul per 112-row tile on the TensorEngine (float32r, full
rate at N=336).  Sum-normalization folds into the x-side factors; peak
normalization folds into the PSUM->SBUF copy as a per-partition scale.

Sharding: pure data parallel over batch B=8 across the 8 cores.
"""

import functools
from contextlib import ExitStack

import numpy as np

try:
    import concourse.bass as bass
except ImportError:  # pragma: no cover
    import sys

    sys.path.insert(0, "/opt/trn_rl_repo")
    import concourse.bass as bass

import concourse.tile as tile
from concourse import bacc, mybir
from concourse.bass_utils import run_bass_kernel_spmd

H = W = 336
P = 112  # partitions per y-chunk; y = 3*p + c  (c in 0..2)
NCH = 3
S_DIM, C_DIM = 8, 4
NMAPS = S_DIM * C_DIM  # 32 maps per core
NR = 2 * NMAPS  # 64 factor rows (map-major, gaze/hand interleaved)
NB = 8  # free blocks in the aligned factor layout (map j = 4*b + q)
N_CORES = 8
SIGMA = 10.0 / 336.0
EXP_SCALE = -1.0 / (2.0 * SIGMA * SIGMA)
EPS = 1e-6
GROUPS = [4, 4, 8, 8, 8]

F32 = mybir.dt.float32
F32R = mybir.dt.float32r
AF = mybir.ActivationFunctionType
ALU = mybir.AluOpType
AX = mybir.AxisListType


def _emit(nc, tc, ctx, ncx_in, ncy_in, out_t, g_const, idt_const, ones_const,
          fy_dram, fx_dram):
    const = ctx.enter_context(tc.tile_pool(name="const", bufs=1))
    fact = ctx.enter_context(tc.tile_pool(name="fact", bufs=1))
    ffac = ctx.enter_context(tc.tile_pool(name="ffac", bufs=1))
    small = ctx.enter_context(tc.tile_pool(name="small", bufs=2))
    stage = ctx.enter_context(tc.tile_pool(name="stage", bufs=2))
    pmap = ctx.enter_context(tc.tile_pool(name="pmap", bufs=2, space="PSUM"))
    ptiny = ctx.enter_context(tc.tile_pool(name="ptiny", bufs=2, space="PSUM"))

    # ---- constants / inputs to SBUF ----
    G = const.tile([NR, W], F32)
    nc.sync.dma_start(G[:], g_const.ap())
    IDT = const.tile([128, 128], F32)
    nc.sync.dma_start(IDT[:], idt_const.ap())
    ONES = const.tile([1, P], F32)
    nc.sync.dma_start(ONES[:], ones_const.ap())
    NCX = const.tile([NR, 1], F32)
    nc.sync.dma_start(NCX[:], ncx_in.ap())
    NCY = const.tile([NR, 1], F32)
    nc.sync.dma_start(NCY[:], ncy_in.ap())
    MBUF = const.tile([128, NMAPS], F32)
    nc.gpsimd.memset(MBUF[:], 0.0)

    # ---- 1-D gaussian factors, dense [64, 336] ----
    sqx = fact.tile([NR, W], F32)
    nc.scalar.activation(sqx[:], G[:], AF.Square, bias=NCX[:, 0:1], scale=1.0)
    fxv = fact.tile([NR, W], F32)
    nc.scalar.activation(fxv[:], sqx[:], AF.Exp, bias=0.0, scale=EXP_SCALE)
    sqy = fact.tile([NR, W], F32)
    nc.scalar.activation(sqy[:], G[:], AF.Square, bias=NCY[:, 0:1], scale=1.0)
    fyv = fact.tile([NR, W], F32)
    nc.scalar.activation(fyv[:], sqy[:], AF.Exp, bias=0.0, scale=EXP_SCALE)

    # sum-normalization scale a = valid / (Sx*Sy + eps), folded into x factors
    sx = small.tile([NR, 1], F32)
    nc.vector.reduce_sum(sx[:], fxv[:], axis=AX.X)
    sy = small.tile([NR, 1], F32)
    nc.vector.reduce_sum(sy[:], fyv[:], axis=AX.X)
    ss = small.tile([NR, 1], F32)
    nc.vector.tensor_mul(ss[:], sx[:], sy[:])
    sse = small.tile([NR, 1], F32)
    nc.vector.tensor_scalar_add(sse[:], ss[:], EPS)
    rec = small.tile([NR, 1], F32)
    nc.vector.reciprocal(rec[:], sse[:])
    vs = small.tile([NR, 1], F32)
    nc.vector.tensor_add(vs[:], NCX[:], NCY[:])
    vm = small.tile([NR, 1], F32)  # valid: cx+cy > 0  <=>  (-cx)+(-cy) < 0
    nc.vector.tensor_scalar(vm[:], vs[:], 0.0, None, op0=ALU.is_lt)
    av = small.tile([NR, 1], F32)
    nc.vector.tensor_mul(av[:], rec[:], vm[:])
    fxs = fact.tile([NR, W], F32)
    nc.vector.tensor_scalar_mul(fxs[:], fxv[:], av[:, 0:1])

    # ---- bounce through DRAM into the 32-aligned pair layout ----
    # FY/FX[32*q + t, b, x] = factor row (2*j + t) with j = 4*b + q
    nc.sync.dma_start(fy_dram.ap(), fyv[:])
    nc.sync.dma_start(fx_dram.ap(), fxs[:])
    FY = ffac.tile([128, NB, W], F32)
    FX = ffac.tile([128, NB, W], F32)
    fyr = fy_dram.ap().rearrange("(b r) x -> r b x", r=8)  # r = 2q+t
    fxr = fx_dram.ap().rearrange("(b r) x -> r b x", r=8)
    for q in range(4):
        nc.sync.dma_start(FY[32 * q:32 * q + 2, :, :], fyr[2 * q:2 * q + 2, :, :])
        nc.sync.dma_start(FX[32 * q:32 * q + 2, :, :], fxr[2 * q:2 * q + 2, :, :])

    # DRAM view matching the stage layout: out[m, y, x] with y = 3*p + c
    dview = out_t.ap().rearrange("m (p c) x -> p m c x", p=P)

    def map_matmuls(j, pt):
        q, b = j % 4, j // 4
        rhs = FX[32 * q:32 * q + 2, b, :].bitcast(F32R)
        for cix in range(NCH):
            lhsT = FY[32 * q:32 * q + 2, b, cix::3].bitcast(F32R)
            nc.tensor.matmul(pt[:, cix * 512:cix * 512 + W], lhsT, rhs,
                             start=True, stop=True, tile_position=(32 * q, 0))

    j0 = 0
    for g in GROUPS:
        # pass A: generate maps, per-partition max into MBUF columns
        for j in range(j0, j0 + g):
            pt = pmap.tile([P, NCH * 512], F32, tag="pmap")
            map_matmuls(j, pt)
            red_in = pt[:].rearrange("p (c z) -> p c z", c=NCH)[:, :, 0:W]
            nc.vector.reduce_max(MBUF[0:P, j:j + 1], red_in, axis=AX.XY)

        # group chain: peaks -> 1/(peak+eps) broadcast to [P, g]
        tp = ptiny.tile([P, 512], F32, tag="ptiny")
        mT = tp[0:g, 0:128]
        nc.tensor.transpose(mT, MBUF[:, j0:j0 + g], IDT[:, :])
        pk = small.tile([8, 1], F32, tag="pk")
        nc.vector.reduce_max(pk[0:g, :], mT, axis=AX.X)
        pke = small.tile([8, 1], F32, tag="pke")
        nc.vector.tensor_scalar_add(pke[0:g, :], pk[0:g, :], EPS)
        rcp = small.tile([8, 1], F32, tag="rcp")
        nc.vector.reciprocal(rcp[0:g, :], pke[0:g, :])
        rrow_p = tp[0:1, 128:128 + g]
        nc.tensor.transpose(rrow_p, rcp[0:g, :], IDT[0:g, 0:g])
        rrow_s = small.tile([1, 8], F32, tag="rrow")
        nc.scalar.copy(rrow_s[0:1, 0:g], rrow_p)
        rg_p = tp[0:P, 256:256 + g]
        nc.tensor.matmul(rg_p, ONES[:, :], rrow_s[0:1, 0:g], start=True, stop=True)
        rg = small.tile([P, 8], F32, tag="rg")
        nc.scalar.copy(rg[:, 0:g], rg_p)

        # pass B: regenerate maps, scaled copy to SBUF stage, DMA out
        st = stage.tile([P, g, NCH, W], F32, tag="stage")
        for j in range(j0, j0 + g):
            gi = j - j0
            pt = pmap.tile([P, NCH * 512], F32, tag="pmap")
            map_matmuls(j, pt)
            cp_in = pt[:].rearrange("p (c z) -> p c z", c=NCH)[:, :, 0:W]
            nc.scalar.activation(st[:, gi, :, :], cp_in, AF.Copy, bias=0.0,
                                 scale=rg[:, gi:gi + 1])
        nc.sync.dma_start(dview[:, j0:j0 + g, :, :], st[:])
        j0 += g


@functools.lru_cache(maxsize=1)
def _build():
    nc = bacc.Bacc("TRN2", target_bir_lowering=False, debug=False)
    ncx_in = nc.dram_tensor("negcx", [NR, 1], F32, kind="ExternalInput")
    ncy_in = nc.dram_tensor("negcy", [NR, 1], F32, kind="ExternalInput")
    out_t = nc.dram_tensor("out", [NMAPS, H, W], F32, kind="ExternalOutput")

    grid = (np.arange(W, dtype=np.float64) / (W - 1)).astype(np.float32)
    g_const = nc.inline_tensor(np.tile(grid, (NR, 1)), name="gridc")
    idt_const = nc.inline_tensor(np.eye(128, dtype=np.float32), name="idtc")
    ones_const = nc.inline_tensor(np.ones((1, P), np.float32), name="onesc")
    fy_dram = nc.dram_tensor("fy_scratch", [NR, W], F32)
    fx_dram = nc.dram_tensor("fx_scratch", [NR, W], F32)

    with tile.TileContext(nc) as tc, ExitStack() as ctx:
        _emit(nc, tc, ctx, ncx_in, ncy_in, out_t, g_const, idt_const,
              ones_const, fy_dram, fx_dram)
    nc.compile()
    return nc


def _in_map_for(gaze, hand, b):
    cg = np.asarray(gaze[b], dtype=np.float32).reshape(NMAPS, 2)
    ch = np.asarray(hand[b], dtype=np.float32).reshape(NMAPS, 2)
    inter = np.stack([cg, ch], axis=1).reshape(NR, 2)  # row 2*j + t
    return {
        "negcx": np.ascontiguousarray(-inter[:, 0:1]),
        "negcy": np.ascontiguousarray(-inter[:, 1:2]),
    }


def kernel(gaze_coords, hand_coords, _trace=False, **trace_kwargs):
    gaze_coords = np.asarray(gaze_coords, dtype=np.float32)
    hand_coords = np.asarray(hand_coords, dtype=np.float32)
    B = gaze_coords.shape[0]
    assert B == N_CORES, f"expected batch {N_CORES}, got {B}"
    nc = _build()
    in_maps = [_in_map_for(gaze_coords, hand_coords, b) for b in range(B)]
    res = run_bass_kernel_spmd(nc, in_maps, list(range(N_CORES)),
                               trace=_trace, **trace_kwargs)
    out = np.stack(
        [res.results[i]["out"].reshape(S_DIM, C_DIM, H, W) for i in range(B)],
        axis=0,
    ).astype(np.float32)
    if _trace:
        return out, res
    return out


# revision 9
# speedup vs baseline: 1.0679x; 1.0679x over previous
"""Trainium2 Bass kernel for nn_HeatmapEncoder.

Math per (b, s, c) and per coordinate set (gaze, hand):
    g = exp(-((gx-cx)^2 + (gy-cy)^2) / (2 sigma^2))   on a 336x336 grid
    g = g / (sum(g) + eps)            (zeroed when cx+cy <= 0)
    unified = g_gaze + g_hand
    out = unified / (max(unified) + eps)

The Gaussian is separable, so each unified map is rank-2.  Each map is
generated by K=6 bf16 matmuls (hi/lo split of each fp32 factor; the
yl*xl term is dropped, rel err ~2^-16):
    rows (per set): (yh, xh), (yh, xl), (yl, xh)
Sum-normalization is folded into the x factors, peak normalization into
the PSUM->SBUF copy (ACT Copy with per-partition scale).

Layout: map j = 4*b + q keeps its 6 factor rows at SBUF partitions
32*q .. 32*q+5, free block b (PE row-tiles are tied to 32-aligned
partition groups).  Map rows are interleaved y = 3*p + c so each map is
a single contiguous DRAM range for the output DMA.

Sharding: pure data parallel over batch B=8 across the 8 cores.
"""

import functools
from contextlib import ExitStack

import numpy as np

try:
    import concourse.bass as bass
except ImportError:  # pragma: no cover
    import sys

    sys.path.insert(0, "/opt/trn_rl_repo")
    import concourse.bass as bass

import concourse.tile as tile
from concourse import bacc, mybir
from concourse.bass_utils import run_bass_kernel_spmd

H = W = 336
P = 112  # partitions per y-chunk; y = 3*p + c  (c in 0..2)
NCH = 3
S_DIM, C_DIM = 8, 4
NMAPS = S_DIM * C_DIM  # 32 maps per core
NR = 2 * NMAPS  # 64 factor rows (map-major, gaze/hand interleaved)
NB = 8  # free blocks in the aligned factor layout (map j = 4*b + q)
N_CORES = 8
SIGMA = 10.0 / 336.0
EXP_SCALE = -1.0 / (2.0 * SIGMA * SIGMA)
EPS = 1e-6
GROUPS = [4, 8, 8, 8, 4]

F32 = mybir.dt.float32
BF16 = mybir.dt.bfloat16
AF = mybir.ActivationFunctionType
ALU = mybir.AluOpType
AX = mybir.AxisListType

# packed constant layout: [128, 640] fp32
#   cols   0:336  rows 0:64   grid (arange(W)/(W-1))
#   cols 336:464  rows 0:128  identity
#   cols 464:576  row  0      ones (broadcast matmul lhsT)
PK_W = 640


def _emit(nc, tc, ctx, negc_in, out_t, pk_const, ystg, xstg):
    const = ctx.enter_context(tc.tile_pool(name="const", bufs=1))
    fact = ctx.enter_context(tc.tile_pool(name="fact", bufs=1))
    ffac = ctx.enter_context(tc.tile_pool(name="ffac", bufs=1))
    small = ctx.enter_context(tc.tile_pool(name="small", bufs=2))
    stage = ctx.enter_context(tc.tile_pool(name="stage", bufs=3))
    pmap = ctx.enter_context(tc.tile_pool(name="pmap", bufs=2, space="PSUM"))
    ptiny = ctx.enter_context(tc.tile_pool(name="ptiny", bufs=2, space="PSUM"))

    # ---- early ACT table preload via dummy exp on a memset tile ----
    dum = small.tile([1, 16], F32, tag="dum")
    nc.gpsimd.memset(dum[:], 0.0)
    dum2 = small.tile([1, 16], F32, tag="dum2")
    nc.scalar.activation(dum2[:], dum[:], AF.Exp, bias=0.0, scale=1.0)

    # ---- constants / inputs ----
    PK = const.tile([128, PK_W], F32)
    nc.sync.dma_start(PK[:], pk_const.ap())
    NC2 = const.tile([NR, 2], F32)
    nc.sync.dma_start(NC2[:], negc_in.ap())
    G = PK[0:NR, 0:W]
    ONES = PK[0:1, 464:464 + P]
    IDT = PK[:, 336:464]
    MBUF = const.tile([128, NMAPS], F32)
    nc.gpsimd.memset(MBUF[:], 0.0)

    # ---- 1-D gaussian factors, dense [64, 336] fp32 (y side first) ----
    sqy = fact.tile([NR, W], F32)
    nc.scalar.activation(sqy[:], G, AF.Square, bias=NC2[:, 1:2], scale=1.0)
    fyv = fact.tile([NR, W], F32)
    nc.scalar.activation(fyv[:], sqy[:], AF.Exp, bias=0.0, scale=EXP_SCALE)
    sqx = fact.tile([NR, W], F32)
    nc.scalar.activation(sqx[:], G, AF.Square, bias=NC2[:, 0:1], scale=1.0)
    fxv = fact.tile([NR, W], F32)
    nc.scalar.activation(fxv[:], sqx[:], AF.Exp, bias=0.0, scale=EXP_SCALE)

    # y-side hi/lo split (unscaled)
    yh = fact.tile([NR, W], BF16)
    nc.scalar.activation(yh[:], fyv[:], AF.Copy, bias=0.0, scale=1.0)
    yl = fact.tile([NR, W], BF16)
    nc.vector.tensor_sub(yl[:], fyv[:], yh[:])

    # normalization scale a = valid / (Sx*Sy + eps) folded into x factors
    sx = small.tile([NR, 1], F32, tag="sx")
    nc.vector.reduce_sum(sx[:], fxv[:], axis=AX.X)
    sy = small.tile([NR, 1], F32, tag="sy")
    nc.vector.reduce_sum(sy[:], fyv[:], axis=AX.X)
    ss = small.tile([NR, 1], F32, tag="ss")
    nc.vector.tensor_mul(ss[:], sx[:], sy[:])
    sse = small.tile([NR, 1], F32, tag="sse")
    nc.vector.tensor_scalar_add(sse[:], ss[:], EPS)
    rec = small.tile([NR, 1], F32, tag="rec")
    nc.vector.reciprocal(rec[:], sse[:])
    vs = small.tile([NR, 1], F32, tag="vs")
    nc.vector.tensor_add(vs[:], NC2[:, 0:1], NC2[:, 1:2])
    vm = small.tile([NR, 1], F32, tag="vm")  # valid: (-cx)+(-cy) < 0
    nc.vector.tensor_scalar(vm[:], vs[:], 0.0, None, op0=ALU.is_lt)
    av = small.tile([NR, 1], F32, tag="av")
    nc.vector.tensor_mul(av[:], rec[:], vm[:])
    fxs = fact.tile([NR, W], F32)
    nc.vector.tensor_scalar_mul(fxs[:], fxv[:], av[:, 0:1])

    # x-side hi/lo split (scaled)
    xh = fact.tile([NR, W], BF16)
    nc.scalar.activation(xh[:], fxs[:], AF.Copy, bias=0.0, scale=1.0)
    xl = fact.tile([NR, W], BF16)
    nc.vector.tensor_sub(xl[:], fxs[:], xh[:])

    # ---- bounce through DRAM into the 32-aligned 6-row layout ----
    # staging [3, 64, 336]: y side (yh, yh, yl); x side (xh, xl, xh)
    nc.sync.dma_start(ystg.ap()[0], yh[:])
    nc.scalar.dma_start(ystg.ap()[1], yh[:])
    nc.sync.dma_start(ystg.ap()[2], yl[:])
    nc.sync.dma_start(xstg.ap()[0], xh[:])
    nc.scalar.dma_start(xstg.ap()[1], xl[:])
    nc.scalar.dma_start(xstg.ap()[2], xh[:])

    FY = ffac.tile([128, NB, W], BF16)
    FX = ffac.tile([128, NB, W], BF16)
    ya = ystg.ap()  # [3, 64, 336]
    xa = xstg.ap()
    for q in range(4):
        for t in range(2):
            # dest rows 32q+3t+u  <-  stg[u, 8b+2q+t, x]
            nc.sync.dma_start(FY[32 * q + 3 * t:32 * q + 3 * t + 3, :, :],
                              ya[:, 2 * q + t::8, :])
            nc.scalar.dma_start(FX[32 * q + 3 * t:32 * q + 3 * t + 3, :, :],
                                xa[:, 2 * q + t::8, :])

    # DRAM view matching stage layout: out[m, y, x], y = 3p+c, z = 336c+x
    dview = out_t.ap().rearrange("m (p c) x -> p m (c x)", p=P)

    def map_matmuls(j, pt):
        q, b = j % 4, j // 4
        rhs = FX[32 * q:32 * q + 6, b, :]
        for cix in range(NCH):
            lhsT = FY[32 * q:32 * q + 6, b, cix::3]
            nc.tensor.matmul(pt[:, cix * 512:cix * 512 + W], lhsT, rhs,
                             start=True, stop=True, tile_position=(32 * q, 0))

    j0 = 0
    for g in GROUPS:
        # pass A: generate maps, per-partition max into MBUF columns
        for j in range(j0, j0 + g):
            pt = pmap.tile([P, NCH * 512], F32, tag="pmap")
            map_matmuls(j, pt)
            red_in = pt[:].rearrange("p (c z) -> p c z", c=NCH)[:, :, 0:W]
            nc.vector.reduce_max(MBUF[0:P, j:j + 1], red_in, axis=AX.XY)

        # group chain: peaks -> 1/(peak+eps) broadcast to [P, g]
        tp = ptiny.tile([P, 512], F32, tag="ptiny")
        mT = tp[0:g, 0:128]
        nc.tensor.transpose(mT, MBUF[:, j0:j0 + g], IDT)
        pk = small.tile([8, 1], F32, tag="pk")
        nc.vector.reduce_max(pk[0:g, :], mT, axis=AX.X)
        pke = small.tile([8, 1], F32, tag="pke")
        nc.vector.tensor_scalar_add(pke[0:g, :], pk[0:g, :], EPS)
        rcp = small.tile([8, 1], F32, tag="rcp")
        nc.vector.reciprocal(rcp[0:g, :], pke[0:g, :])
        rrow_p = tp[0:1, 128:128 + g]
        nc.tensor.transpose(rrow_p, rcp[0:g, :], IDT[0:g, 0:g])
        rrow_s = small.tile([1, 8], F32, tag="rrow")
        nc.scalar.copy(rrow_s[0:1, 0:g], rrow_p)
        rg_p = tp[0:P, 256:256 + g]
        nc.tensor.matmul(rg_p, ONES, rrow_s[0:1, 0:g], start=True, stop=True)
        rg = small.tile([P, 8], F32, tag="rg")
        nc.scalar.copy(rg[:, 0:g], rg_p)

        # pass B: regenerate maps, scaled copy to stage, DMA out per 2 maps
        for j0p in range(j0, j0 + g, 2):
            st = stage.tile([P, 2, NCH * W], F32, tag="stage")
            for j in (j0p, j0p + 1):
                gi, si = j - j0, j - j0p
                pt = pmap.tile([P, NCH * 512], F32, tag="pmap")
                map_matmuls(j, pt)
                cp_in = pt[:].rearrange("p (c z) -> p c z", c=NCH)[:, :, 0:W]
                cp_out = st[:, si, :].rearrange("p (c x) -> p c x", c=NCH)
                nc.scalar.activation(cp_out, cp_in, AF.Copy, bias=0.0,
                                     scale=rg[:, gi:gi + 1])
            nc.sync.dma_start(dview[:, j0p:j0p + 2, :], st[:])
        j0 += g


@functools.lru_cache(maxsize=1)
def _build():
    nc = bacc.Bacc("TRN2", target_bir_lowering=False, debug=False)
    negc_in = nc.dram_tensor("negc", [NR, 2], F32, kind="ExternalInput")
    out_t = nc.dram_tensor("out", [NMAPS, H, W], F32, kind="ExternalOutput")

    pk = np.zeros((128, PK_W), np.float32)
    grid = (np.arange(W, dtype=np.float64) / (W - 1)).astype(np.float32)
    pk[0:NR, 0:W] = grid[None, :]
    pk[0, 464:464 + P] = 1.0
    pk[:, 336:464] = np.eye(128, dtype=np.float32)
    pk_const = nc.inline_tensor(pk, name="pkc")

    ystg = nc.dram_tensor("ystg", [3, NR, W], BF16)
    xstg = nc.dram_tensor("xstg", [3, NR, W], BF16)

    with tile.TileContext(nc) as tc, ExitStack() as ctx:
        _emit(nc, tc, ctx, negc_in, out_t, pk_const, ystg, xstg)
    nc.compile()
    return nc


def _in_map_for(gaze, hand, b):
    cg = np.asarray(gaze[b], dtype=np.float32).reshape(NMAPS, 2)
    ch = np.asarray(hand[b], dtype=np.float32).reshape(NMAPS, 2)
    inter = np.stack([cg, ch], axis=1).reshape(NR, 2)  # row 2*j + t
    return {"negc": np.ascontiguousarray(-inter)}


def kernel(gaze_coords, hand_coords, _trace=False, **trace_kwargs):
    gaze_coords = np.asarray(gaze_coords, dtype=np.float32)
    hand_coords = np.asarray(hand_coords, dtype=np.float32)
    B = gaze_coords.shape[0]
    assert B == N_CORES, f"expected batch {N_CORES}, got {B}"
    nc = _build()
    in_maps = [_in_map_for(gaze_coords, hand_coords, b) for b in range(B)]
    res = run_bass_kernel_spmd(nc, in_maps, list(range(N_CORES)),
                               trace=_trace, **trace_kwargs)
    out = np.stack(
        [res.results[i]["out"].reshape(S_DIM, C_DIM, H, W) for i in range(B)],
        axis=0,
    ).astype(np.float32)
    if _trace:
        return out, res
    return out
